# revision 16
# baseline (speedup 1.0000x reference)
"""Trainium2 Bass kernel for MessagePassingEdgeModule.

reference:
    src, dst = edge_index
    agg = concat([x[src], x[dst], edge_attr, u[batch[src]]], axis=1)  # [E, 512]
    h   = relu(agg @ W1 + b1)                                         # [E, 128]
    out = relu(h @ W2 + b2)                                           # [E, 128]

Strategy (8 cores, edge-parallel):
  - Nodes split in two halves (< / >= 25000); edges bucketed by
    (src_half, dst_half), 2 cores per bucket.
  - src side needs NO gather descriptors: each core's edges are binned by
    src band (128 consecutive node ids). Tile t of group g holds only edges
    with src in band 8g+t, so the per-tile "gather" is a one-hot expansion
    matmul on the PE against a per-band table pb computed on the fly:
        pb[node, h] = x[node] @ W1s + Wu[batch[node]]   (u folded in, so
        the per-edge u one-hot pass and its 2.9MB stream are gone)
        h_tile += pb^T @ S,  S[p,e] = (srcoff[e] & 127 == p)
    S is built from a K=1 broadcast matmul of the host-streamed
    (srcoff&127) row + a DVE is_equal against an iota column.
  - dst side keeps ONE dma_gather per group (the unavoidable random side)
    from an SBUF-resident half-table (row n -> partition n%128, stripe
    n//128), elem 256B. Splitting it (even across SWDGE queues) measured
    slower/broken: queue 1 completions are not synchronized by the Tile
    framework, and a second same-queue gather costs ~5µs/instr.
  - src table (feature-major) and dst-gather indices are SBUF-resident,
    loaded once outside the loop (saves ~26µs/iter of per-group DMA).
  - Tile-level software pipelining: stage A (S-build, pb) of tile k+1 is
    emitted before stage B (h-accumulate, relus, W2) of tile k; group loads
    prefetch LEAD=4 tiles early. relu2 runs on DVE for 2 of 8 tiles per
    group to offload the Act engine.
  - edge_attr streamed feature-major f16 (fp8 works at absmax 1.45e-2 but
    only saves ~3% — not worth the error budget), output written
    feature-major f16 and un-permuted on host.
Measured (slope method, device-resident inputs): 741us/iter vs 892us for
the session-start baseline; DMA is the wall (gather ~289us, stores ~190us,
ea stream ~165us; streams/stores/gather do NOT overlap on hardware).
"""
import sys
if '/opt/trn_rl_repo' not in sys.path:
    sys.path.insert(0, '/opt/trn_rl_repo')

from contextlib import ExitStack

import numpy as np

import concourse.bass as bass
import concourse.mybir as mybir
import concourse.tile as tile
from concourse import bacc
from concourse.bass_utils import run_bass_kernel_spmd

N_NODES = 50000
N_EDGES = 640000
N_GRAPHS = 16
D = 128
N_CORES = 8
SPLIT = 25000            # node-id split for the two half-tables
TBL_RANKS = 196          # ceil(SPLIT/128) stripes for the SBUF dst table
TBL_PAD = TBL_RANKS * 128            # 25088
SRC_ROWS = 25600         # src table rows (200 bands of 128)

TILE_E = 448             # edge slots per tile (= per src band)
GROUP_TILES = 8
GROUP_E = TILE_E * GROUP_TILES       # 3584
N_GROUPS = 25            # 25 groups x 8 bands = 200 bands (196 used)
SLOTS = N_GROUPS * GROUP_E           # 89600 slots per core
BAND = 128
GBAND = GROUP_TILES * BAND           # 1024 src-table rows per group

f32 = mybir.dt.float32
f16 = mybir.dt.float16
i16 = mybir.dt.int16

_CACHE = {}
import os as _os
FP8_EA = _os.environ.get("K_FP8_EA") == "1"
GQ = int(_os.environ.get("K_GQ", "1"))   # dst gather split factor
GQ1 = _os.environ.get("K_GQ1") == "1"    # force all splits onto queue 0



def _wrap_idx(seq):
    """[..., n] -> [..., 128, n//16] int16 in dma_gather's wrapped layout."""
    n = seq.shape[-1]
    lead = seq.shape[:-1]
    w = seq.reshape(*lead, n // 16, 16)
    w = np.swapaxes(w, -1, -2)
    return np.tile(w, (*(1 for _ in lead), 8, 1)).astype(np.int16)


def _build_program(reps: int = 1, skip_gathers=False, skip_compute=False,
                   skip_stores=False, hbm_dst=False, sb_bufs=3, hp_bufs=4,
                   pe_bc=True, dve_relu2=2, resident_idx=True, fp8_ea=None,
                   pipeline=True, store_act=False, ea_act=False):
    nc = bacc.Bacc("TRN2", target_bir_lowering=False, debug=False,
                   num_devices=N_CORES, num_swdge_queues=1 if GQ1 else GQ)

    tbsT_d = nc.dram_tensor("tbsT", [D, SRC_ROWS], f16, kind="ExternalInput").ap()
    tbd_d = nc.dram_tensor("tbd", [TBL_PAD, D], f16, kind="ExternalInput").ap()
    if fp8_ea is None:
        fp8_ea = FP8_EA
    ea_dt = mybir.dt.float8e4 if fp8_ea else f16
    ea_d = nc.dram_tensor("ea", [D, SLOTS], ea_dt, kind="ExternalInput").ap()
    ohn_d = nc.dram_tensor("ohn", [N_GROUPS, 16, GBAND], f16,
                           kind="ExternalInput").ap()
    sm_d = nc.dram_tensor("sm", [1, SLOTS], f16, kind="ExternalInput").ap()
    idx_d = nc.dram_tensor("idx", [128, N_GROUPS, GROUP_E // 16], i16,
                           kind="ExternalInput").ap()
    iota_d = nc.dram_tensor("iota", [128, 1], f32, kind="ExternalInput").ap()
    w1_d = nc.dram_tensor("w1", [4 * D, D], f32, kind="ExternalInput").ap()
    w2_d = nc.dram_tensor("w2", [D, D], f32, kind="ExternalInput").ap()
    b1_d = nc.dram_tensor("b1", [D], f32, kind="ExternalInput").ap()
    b2_d = nc.dram_tensor("b2", [D], f32, kind="ExternalInput").ap()
    u_d = nc.dram_tensor("u", [N_GRAPHS, D], f32, kind="ExternalInput").ap()
    out_d = nc.dram_tensor("out", [D, SLOTS], f16, kind="ExternalOutput").ap()

    IW = GROUP_E // 16   # dst idx cols per group (224)

    with tile.TileContext(nc) as tc, ExitStack() as ctx:
        const = ctx.enter_context(tc.tile_pool(name="const", bufs=1))
        sb = ctx.enter_context(tc.tile_pool(name="sb", bufs=sb_bufs))
        hp = ctx.enter_context(tc.tile_pool(name="hp", bufs=hp_bufs))
        op = ctx.enter_context(tc.tile_pool(name="op", bufs=2))
        ps = ctx.enter_context(tc.tile_pool(name="ps", bufs=2, space="PSUM"))

        # ---- constants ----
        from concourse.masks import make_identity
        ident = const.tile([128, 128], f32)
        make_identity(nc, ident[:])

        w1_sb = const.tile([128, 4, D], f32)
        for c in range(4):
            nc.sync.dma_start(w1_sb[:, c, :], w1_d[c * D:(c + 1) * D, :])
        w2_f32 = const.tile([128, D], f32)
        nc.sync.dma_start(w2_f32[:], w2_d[:])
        w1f = const.tile([128, 4, D], f16)
        nc.vector.tensor_copy(w1f[:], w1_sb[:])
        w2f = const.tile([128, D], f16)
        nc.vector.tensor_copy(w2f[:], w2_f32[:])

        b1c = const.tile([128, 1], f32)
        nc.sync.dma_start(b1c[:], b1_d[:].rearrange("(p one) -> p one", one=1))
        b2c = const.tile([128, 1], f32)
        nc.sync.dma_start(b2c[:], b2_d[:].rearrange("(p one) -> p one", one=1))
        iota_sb = const.tile([128, 1], f32)
        nc.sync.dma_start(iota_sb[:], iota_d[:])
        ones1 = const.tile([1, 128], f16)
        nc.vector.memset(ones1[:], 1.0)

        # Wu = u @ W1u  ([16, 128])
        u_sb = const.tile([16, D], f32)
        nc.sync.dma_start(u_sb[:], u_d[:])
        ut_ps = ps.tile([128, 16], f32, tag="h")
        nc.tensor.transpose(out=ut_ps[:], in_=u_sb[:], identity=ident[:16, :16])
        ut_sb = const.tile([128, 16], f32)
        nc.vector.tensor_copy(ut_sb[:], ut_ps[:])
        wu_ps = ps.tile([16, 128], f32, tag="o")
        nc.tensor.matmul(wu_ps[:], ut_sb[:], w1_sb[:, 3, :], start=True,
                         stop=True)
        wu_sb = const.tile([16, 128], f16)
        nc.vector.tensor_copy(wu_sb[:], wu_ps[:])

        # src table resident, feature-major: [feat=128, SRC_ROWS]
        xs_all = const.tile([128, SRC_ROWS], f16)
        nc.sync.dma_start(xs_all[:], tbsT_d[:])

        # dst-gather indices resident: [128, N_GROUPS, IW]
        if resident_idx:
            idx_all = const.tile([128, N_GROUPS, IW], i16)
            nc.sync.dma_start(idx_all[:], idx_d[:])

        # dst half-table resident in SBUF: row n -> partition n%128,
        # stripe n//128 (dma_gather sbuf_tokens_per_rank=128 layout)
        if not hbm_dst:
            tbd_sb = const.tile([128, TBL_RANKS, D], f16)
            nc.sync.dma_start(tbd_sb[:],
                              tbd_d[:].rearrange("(s p) k -> p s k", p=128))

        # ---- main loop ----
        # Software-pipelined at tile granularity: stage A (bc, S-build, pb)
        # of tile k+1 is emitted before stage B (h-accumulate, relus, W2) of
        # tile k, so each engine's in-order stream always has a tile of
        # slack on cross-engine dependencies.
        def emit_group_loads(g, state):
            if resident_idx:
                idx_sb = idx_all[:, g, :]
            else:
                idx_t = sb.tile([128, IW], i16, tag="idx")
                nc.sync.dma_start(idx_t[:], idx_d[:, g, :])
                idx_sb = idx_t[:]

            sm_sb = sb.tile([1, GROUP_E], f16, tag="sm")
            nc.sync.dma_start(sm_sb[:], sm_d[:, g * GROUP_E:(g + 1) * GROUP_E])
            ohn_sb = sb.tile([16, GBAND], f16, tag="ohn")
            nc.sync.dma_start(ohn_sb[:], ohn_d[g])

            smB = None
            if not pe_bc:
                smB = sb.tile([128, GROUP_E], f16, tag="smB")
                nc.gpsimd.partition_broadcast(smB[:], sm_sb[:])

            dstT = sb.tile([128, 1, GROUP_E], f16, tag="dstT")
            if skip_gathers:
                nc.vector.memset(
                    dstT[:].rearrange("p a b -> p (a b)"), 0.5)
            elif hbm_dst:
                nc.gpsimd.dma_gather(
                    dstT[:], tbd_d[:], idx_sb, GROUP_E, GROUP_E,
                    D, transpose=True, single_packet=False)
            else:
                ne = GROUP_E // GQ
                iwq = ne // 16
                for q in range(GQ):
                    if resident_idx:
                        idx_q = idx_all[:, g, q * iwq:(q + 1) * iwq]
                    else:
                        idx_q = idx_t[:, q * iwq:(q + 1) * iwq]
                    nc.gpsimd.dma_gather(
                        dstT[:, :, q * ne:(q + 1) * ne], tbd_sb[:],
                        idx_q, ne, ne, D, transpose=True, single_packet=False,
                        sbuf_tokens_per_rank=128,
                        sbuf_free_dim_per_rank=2 * D, queue_num=0 if GQ1 else q)

            eaT = sb.tile([128, GROUP_E], ea_dt, tag="eaT")
            ea_eng = nc.scalar if ea_act else nc.sync
            ea_eng.dma_start(eaT[:], ea_d[:, g * GROUP_E:(g + 1) * GROUP_E])

            outT = op.tile([128, GROUP_E], f16, tag="outT")
            state[g] = dict(sm=sm_sb, smB=smB, ohn=ohn_sb, dstT=dstT,
                            eaT=eaT, outT=outT)

        def tile_front(g, t, st):
            sl = slice(t * TILE_E, (t + 1) * TILE_E)
            # S[p, e] = (srcoff[e] & 127 == p)
            S_sb = hp.tile([128, TILE_E], f16, tag="S")
            if pe_bc:
                bc_ps = ps.tile([128, TILE_E], f32, tag="bc")
                nc.tensor.matmul(bc_ps[:], ones1[:], st["sm"][:, sl],
                                 start=True, stop=True)
                nc.vector.tensor_scalar(
                    S_sb[:], bc_ps[:], iota_sb[:], None,
                    mybir.AluOpType.is_equal)
            else:
                nc.vector.tensor_scalar(
                    S_sb[:], st["smB"][:, sl], iota_sb[:], None,
                    mybir.AluOpType.is_equal)
            # pb[node, h] = x[node] @ W1s + Wu[batch[node]]
            pb_ps = ps.tile([128, BAND], f32, tag="x")
            nc.tensor.matmul(
                pb_ps[:],
                xs_all[:, g * GBAND + t * BAND:g * GBAND + (t + 1) * BAND],
                w1f[:, 0, :], start=True, stop=False)
            nc.tensor.matmul(
                pb_ps[:], st["ohn"][:, t * BAND:(t + 1) * BAND],
                wu_sb[:], start=False, stop=True)
            pb_sb = hp.tile([128, BAND], f16, tag="srcT")
            nc.vector.tensor_copy(pb_sb[:], pb_ps[:])
            return S_sb, pb_sb

        def tile_back(g, t, st, S_sb, pb_sb):
            sl = slice(t * TILE_E, (t + 1) * TILE_E)
            h_ps = ps.tile([128, TILE_E], f32, tag="h")
            nc.tensor.matmul(h_ps[:], w1f[:, 2, :], st["eaT"][:, sl],
                             start=True, stop=False)
            nc.tensor.matmul(h_ps[:], w1f[:, 1, :], st["dstT"][:, 0, sl],
                             start=False, stop=False)
            nc.tensor.matmul(h_ps[:], pb_sb[:], S_sb[:],
                             start=False, stop=True)

            hT = hp.tile([128, TILE_E], f16, tag="hT")
            nc.scalar.activation(hT[:], h_ps[:],
                                 mybir.ActivationFunctionType.Relu,
                                 bias=b1c[:])

            o_ps = ps.tile([128, TILE_E], f32, tag="o")
            nc.tensor.matmul(o_ps[:], w2f[:], hT[:], start=True, stop=True)
            if t < dve_relu2:
                nc.vector.tensor_scalar(
                    st["outT"][:, sl], o_ps[:], b2c[:], 0.0,
                    mybir.AluOpType.add, mybir.AluOpType.max)
            else:
                nc.scalar.activation(
                    st["outT"][:, sl], o_ps[:],
                    mybir.ActivationFunctionType.Relu,
                    bias=b2c[:])

        def store_group(g, st):
            st_eng = nc.scalar if store_act else nc.sync
            if skip_stores:
                st_eng.dma_start(out_d[:, g * GROUP_E:g * GROUP_E + 128],
                                 st["outT"][:, :128])
            else:
                st_eng.dma_start(out_d[:, g * GROUP_E:(g + 1) * GROUP_E],
                                 st["outT"][:])

        LEAD = 4  # prefetch next group's loads this many tiles early

        def emit_main():
            state = {}
            if skip_compute:
                for g in range(N_GROUPS):
                    emit_group_loads(g, state)
                    st = state[g]
                    nc.vector.tensor_add(st["outT"][:, :GROUP_E],
                                         st["dstT"][:, 0, :],
                                         st["dstT"][:, 0, :] if fp8_ea
                                         else st["eaT"][:])
                    store_group(g, st)
                return
            total = N_GROUPS * GROUP_TILES
            emit_group_loads(0, state)
            pend = None
            for k in range(total + 1):
                if k < total:
                    g, t = divmod(k, GROUP_TILES)
                    if t == LEAD and g + 1 < N_GROUPS:
                        emit_group_loads(g + 1, state)
                    cur = (g, t, state[g]) + tile_front(g, t, state[g])
                else:
                    cur = None
                if not pipeline:
                    pend = cur
                    cur = None
                if pend is not None:
                    pg, pt, pst, S_sb, pb_sb = pend
                    tile_back(pg, pt, pst, S_sb, pb_sb)
                    if pt == GROUP_TILES - 1:
                        store_group(pg, pst)
                        del state[pg]
                pend = cur

        if reps == 1:
            emit_main()
        else:
            with tc.For_i(0, reps, 1):
                emit_main()

    nc.compile()
    return nc


def _prep_inputs(x, edge_attr, u, W1, b1, W2, b2, edge_index, batch):
    src = np.asarray(edge_index[0]).astype(np.int64)
    dst = np.asarray(edge_index[1]).astype(np.int64)
    batch = np.asarray(batch).astype(np.int64)

    x16 = np.asarray(x, np.float16)
    tbs0 = np.zeros((SRC_ROWS, D), np.float16)
    tbs0[:SPLIT] = x16[:SPLIT]
    tbs1 = np.zeros((SRC_ROWS, D), np.float16)
    tbs1[:N_NODES - SPLIT] = x16[SPLIT:]
    tbd0 = np.zeros((TBL_PAD, D), np.float16)
    tbd0[:SPLIT] = x16[:SPLIT]
    tbd1 = np.zeros((TBL_PAD, D), np.float16)
    tbd1[:N_NODES - SPLIT] = x16[SPLIT:]
    tbs0T = np.ascontiguousarray(tbs0.T)
    tbs1T = np.ascontiguousarray(tbs1.T)

    # per-node one-hot of batch id for the two src halves (pad rows -> 0;
    # harmless: S never selects pad rows)
    nb0 = np.zeros(SRC_ROWS, np.int64)
    nb0[:SPLIT] = batch[:SPLIT]
    nb1 = np.zeros(SRC_ROWS, np.int64)
    nb1[:N_NODES - SPLIT] = batch[SPLIT:]
    ohn0 = (np.arange(16)[:, None] == nb0[None, :]).astype(np.float16)
    ohn1 = (np.arange(16)[:, None] == nb1[None, :]).astype(np.float16)
    # [N_GROUPS, 16, GBAND]
    ohn0 = np.ascontiguousarray(
        ohn0.reshape(16, N_GROUPS, GBAND).transpose(1, 0, 2))
    ohn1 = np.ascontiguousarray(
        ohn1.reshape(16, N_GROUPS, GBAND).transpose(1, 0, 2))

    srcoff = np.where(src >= SPLIT, src - SPLIT, src)
    dstoff = np.where(dst >= SPLIT, dst - SPLIT, dst)
    bucket = (src >= SPLIT) * 2 + (dst >= SPLIT)
    band = srcoff >> 7                          # 0..195

    key = bucket * 256 + band
    order = np.argsort(key, kind="stable")
    cnt = np.bincount(key, minlength=4 * 256).reshape(4, 256)

    perm = np.full((N_CORES, SLOTS), -1, np.int64)
    overflow = []
    pos = 0
    for b in range(4):
        for bd in range(256):
            n = cnt[b, bd]
            if n == 0:
                continue
            ids = order[pos:pos + n]
            pos += n
            n0 = min((n + 1) // 2, TILE_E)
            n1 = min(n - n0, TILE_E)
            if n0 + n1 < n:
                overflow.append(ids[n0 + n1:])
            g, t = bd // GROUP_TILES, bd % GROUP_TILES
            base = g * GROUP_E + t * TILE_E
            perm[2 * b, base:base + n0] = ids[:n0]
            perm[2 * b + 1, base:base + n1] = ids[n0:n0 + n1]
    overflow = np.concatenate(overflow) if overflow else np.zeros(0, np.int64)

    ea16 = np.asarray(edge_attr, np.float16)
    W1f = np.asarray(W1, np.float32)
    W2f = np.asarray(W2, np.float32)
    b1f = np.asarray(b1, np.float32)
    b2f = np.asarray(b2, np.float32)
    uf = np.asarray(u, np.float32)
    iota = np.arange(128, dtype=np.float32).reshape(128, 1)

    in_maps = []
    for c in range(N_CORES):
        bs, bd_half = (c // 2) >> 1, (c // 2) & 1
        pc = perm[c]
        valid = pc >= 0
        pv = np.where(valid, pc, 0)

        didx = np.where(valid, dstoff[pv], 0).astype(np.int16)
        sm = np.where(valid, srcoff[pv] & 127, 300).astype(np.float16)
        eac = ea16[pv]
        eac[~valid] = 0

        ea_send = np.ascontiguousarray(eac.T)
        if FP8_EA:
            ea_send = ea_send.astype(mybir.dt.np(mybir.dt.float8e4))
        im = {
            "tbsT": tbs1T if bs else tbs0T,
            "tbd": tbd1 if bd_half else tbd0,
            "ea": ea_send,
            "ohn": ohn1 if bs else ohn0,
            "sm": np.ascontiguousarray(sm.reshape(1, SLOTS)),
            "idx": np.ascontiguousarray(
                _wrap_idx(didx.reshape(N_GROUPS, GQ, GROUP_E // GQ))
                .transpose(2, 0, 1, 3).reshape(128, N_GROUPS, GROUP_E // 16)),
            "iota": iota,
            "w1": W1f, "w2": W2f, "b1": b1f, "b2": b2f, "u": uf,
        }
        in_maps.append(im)
    return in_maps, perm.reshape(-1), overflow


def _cpu_edges(ids, x, edge_attr, u, W1, b1, W2, b2, src, dst, batch):
    agg = np.concatenate([x[src[ids]], x[dst[ids]], edge_attr[ids],
                          u[batch[src[ids]]]], axis=1).astype(np.float32)
    h = np.maximum(agg @ W1 + b1, 0)
    return np.maximum(h @ W2 + b2, 0)


def kernel(x, edge_attr, u, W1, b1, W2, b2, edge_index, batch):
    if "nc" not in _CACHE:
        _CACHE["nc"] = _build_program()
    nc = _CACHE["nc"]
    in_maps, perm, overflow = _prep_inputs(x, edge_attr, u, W1, b1, W2, b2,
                                           edge_index, batch)
    res = run_bass_kernel_spmd(nc, in_maps, list(range(N_CORES)))
    outT = np.concatenate([r["out"] for r in res.results], axis=1)

    out = np.zeros((N_EDGES, D), np.float32)
    valid = perm >= 0
    out[perm[valid]] = outT.T[valid].astype(np.float32)
    if len(overflow):
        src = np.asarray(edge_index[0]).astype(np.int64)
        dst = np.asarray(edge_index[1]).astype(np.int64)
        out[overflow] = _cpu_edges(
            overflow, np.asarray(x, np.float32),
            np.asarray(edge_attr, np.float32),
            np.asarray(u, np.float32), np.asarray(W1, np.float32),
            np.asarray(b1, np.float32), np.asarray(W2, np.float32),
            np.asarray(b2, np.float32), src, dst,
            np.asarray(batch).astype(np.int64))
    return out
